# revision 2
# baseline (speedup 1.0000x reference)
"""Trainium2 Bass kernel for nn_Network_31688268710403 (embedding_lookup).

Computes, for Np=262144 query points, a 3-plane tensorized pose-conditioned
feature lookup:
  - top-1 key-pose retrieval by quaternion similarity selects one pose's
    coord/feat line tables (host-side parameter selection),
  - per-axis 1-D linear grid_sample of the 48-component coord lines,
  - per-plane products + einsum with the 20-dim feat lines,
  - top-k weighted sum (k=1, weight folded into the output copy).

Sharding: data-parallel over points across the 8 NeuronCores; the small
selected parameter tables are replicated to every core.

Device pipeline per core (N=32768 points, chunks of 4096):
  - DVE computes grid positions / lerp fractions,
  - a PE broadcast + DVE pipeline produces int16 gather indices in the
    wrap-16 layout dma_gather expects,
  - GPSIMD dma_gather fetches, per point and axis, one 512 B row-pair
    (table rows i0, i0+1, each padded to 64 floats) from the replicated
    table in HBM,
  - DVE lerps and forms the 3 plane products,
  - PE transposes each 128-point product block and contracts with the
    plane feature matrices into PSUM,
  - results stream out row-major [N, 60].
"""
import sys

try:
    import concourse.bass as _b  # noqa: F401
except ImportError:
    sys.path.insert(0, "/opt/trn_rl_repo")

import numpy as np
import concourse.bass as bass
import concourse.bacc as bacc
import concourse.mybir as mybir
from concourse.tile import TileContext
from concourse import masks
from concourse import bass_utils

F32 = mybir.dt.float32
I16 = mybir.dt.int16
AF = mybir.ActivationFunctionType
OP = mybir.AluOpType

GRID = 512
NCOMP = 48
MAT_MODE = [(0, 1), (0, 2), (1, 2)]
N_CORES = 8
W = 32                      # points per partition per chunk; chunk = 128*W


# ---------------- host-side parameter selection / layout prep ----------------

def _axis_angle_to_quaternion(aa):
    angles = np.linalg.norm(aa, axis=-1, keepdims=True)
    half = angles * 0.5
    small = np.abs(angles) < 1e-6
    safe = np.where(small, 1.0, angles)
    sfac = np.where(small, 0.5 - angles * angles / 48.0, np.sin(half) / safe)
    return np.concatenate([np.cos(half), aa * sfac], axis=-1)


def _topk_select(poses, key_poses, part_indices):
    qp = poses.reshape(poses.shape[0], -1, 3)[:, part_indices]
    kp = key_poses.reshape(key_poses.shape[0], -1, 3)[:, part_indices]
    qq = _axis_angle_to_quaternion(qp.astype(np.float64))
    kq = _axis_angle_to_quaternion(kp.astype(np.float64))
    pose_dist = np.abs((qq[:, None] * kq[None]).sum(-1)).sum(-1)
    sel = int(np.argmax(pose_dist[0]))
    w = float(pose_dist[0, sel])
    w = w / max(abs(w), 1e-16)
    return sel, w


def _pack_tables(coord_line, feat_line, sel):
    tabs = []
    for i in range(3):
        t = np.zeros((GRID + 1, 64), np.float32)
        t[:GRID, :NCOMP] = coord_line[i, sel, :, :, 0].T
        tabs.append(t)
    featc = np.zeros((NCOMP, 64), np.float32)
    for pi in range(3):
        featc[:, 20 * pi:20 * pi + 20] = feat_line[pi, sel, :, :, 0]
    return tabs, featc


def _pos_affine(tbounds):
    b0 = np.asarray(tbounds, np.float64)[0, 0]
    b1 = np.asarray(tbounds, np.float64)[0, 1]
    scales = (GRID - 1) / (b1 - b0)
    offsets = -b0 * scales
    return [float(s) for s in scales], [float(o) for o in offsets]


# ---------------- device kernel ----------------

def _build_kernel(N, scales, offsets, wq):
    NI = 128 * W
    NCH = N // NI
    assert N % NI == 0

    nc = bacc.Bacc()
    pts = nc.dram_tensor("pts", [N, 3], F32, kind="ExternalInput")
    tabs = [nc.dram_tensor(f"tab{i}", [GRID + 1, 64], F32, kind="ExternalInput")
            for i in range(3)]
    featc = nc.dram_tensor("featc", [NCOMP, 64], F32, kind="ExternalInput")
    bc16t = nc.dram_tensor("bc16", [16, 128], F32, kind="ExternalInput")
    outR = nc.dram_tensor("outR", [N, 60], F32, kind="ExternalOutput")
    # overlapping row-pair view for the gather: index g reads 128 floats
    # starting at element 64*g (rows g and g+1 of the padded table)
    tab_gap = [bass.AP(t.ap().tensor, 0, [[64, GRID], [1, 128]]) for t in tabs]

    with TileContext(nc) as tc:
        with (
            tc.tile_pool(name="const", bufs=1) as cpool,
            tc.tile_pool(name="ptsp", bufs=2) as ptsp,
            tc.tile_pool(name="posp", bufs=2) as posp,
            tc.tile_pool(name="gp", bufs=2) as gp,
            tc.tile_pool(name="vp", bufs=1) as vp,
            tc.tile_pool(name="dp", bufs=1) as dp,
            tc.tile_pool(name="prodp", bufs=1) as prodp,
            tc.tile_pool(name="ptp", bufs=4) as ptp,
            tc.tile_pool(name="outp", bufs=2) as outp,
            tc.tile_pool(name="psA", bufs=2, space="PSUM") as psA,
            tc.tile_pool(name="psO", bufs=3, space="PSUM") as psO,
            tc.tile_pool(name="psB", bufs=1, space="PSUM") as psB,
        ):
            ident = cpool.tile([128, 128], F32)
            masks.make_identity(nc, ident[:])
            feat_sb = cpool.tile([NCOMP, 64], F32)
            nc.sync.dma_start(feat_sb[:], featc.ap())
            bc16 = cpool.tile([16, 128], F32)
            nc.sync.dma_start(bc16[:], bc16t.ap())

            MAGIC = float(2 ** 23)
            for ch in range(NCH):
                base = ch * NI
                # A-layout: point j = s*128 + p at [p, s] (matches gather out)
                ptile = ptsp.tile([128, W * 3], F32, tag="ptile")
                nc.sync.dma_start(
                    ptile[:].rearrange("p (s d) -> p s d", d=3),
                    pts.ap()[base:base + NI, :].rearrange("(s p) d -> p s d",
                                                          p=128),
                )
                p3 = ptile[:].rearrange("p (s d) -> p s d", d=3)
                # B-layout (for gather indices): point j = 16c + r at [r, c],
                # replicated into all 8 16-partition groups via a PE broadcast
                ptB16 = ptsp.tile([16, (NI // 16) * 3], F32, tag="ptB16")
                nc.sync.dma_start(
                    ptB16[:].rearrange("r (c d) -> r c d", d=3),
                    pts.ap()[base:base + NI, :].rearrange("(c r) d -> r c d",
                                                          r=16),
                )
                HB = (NI // 16) * 3 // 2
                pbc = []
                for hh in range(2):
                    t = psB.tile([128, HB], F32, tag=f"pbc{hh}")
                    nc.tensor.matmul(t[:], bc16[:],
                                     ptB16[:, hh * HB:(hh + 1) * HB],
                                     start=True, stop=True)
                    pbc.append(t)
                ptB = ptsp.tile([128, (NI // 16) * 3], F32, tag="ptB")
                nc.vector.tensor_copy(ptB[:, 0:HB], pbc[0][:])
                nc.vector.tensor_copy(ptB[:, HB:2 * HB], pbc[1][:])
                pB3 = ptB[:].rearrange("r (c d) -> r c d", d=3)

                vts = []
                for i in range(3):
                    # A-side: lerp fraction f
                    pos = posp.tile([128, W], F32, tag=f"pos{i}")
                    nc.vector.tensor_scalar(pos[:], p3[:, :, i],
                                            float(scales[i]), float(offsets[i]),
                                            OP.mult, OP.add)
                    nc.vector.tensor_scalar(pos[:], pos[:], 0.0,
                                            float(GRID - 1), OP.max, OP.min)
                    # floor via the 2^23 magic-number trick (rounding-mode
                    # independent thanks to the is_gt correction)
                    i0f = posp.tile([128, W], F32, tag=f"i0f{i}")
                    nc.vector.tensor_scalar(i0f[:], pos[:], MAGIC, MAGIC,
                                            OP.add, OP.subtract)
                    gt = posp.tile([128, W], F32, tag=f"gt{i}")
                    nc.vector.tensor_tensor(gt[:], i0f[:], pos[:], OP.is_gt)
                    nc.vector.tensor_tensor(i0f[:], i0f[:], gt[:], OP.subtract)
                    f = posp.tile([128, W], F32, tag=f"f{i}")
                    nc.vector.tensor_tensor(f[:], pos[:], i0f[:], OP.subtract)

                    # B-side: int16 gather indices in wrap-16 layout
                    posB = posp.tile([128, NI // 16], F32, tag=f"posB{i}")
                    nc.vector.tensor_scalar(posB[:], pB3[:, :, i],
                                            float(scales[i]), float(offsets[i]),
                                            OP.mult, OP.add)
                    nc.vector.tensor_scalar(posB[:], posB[:], 0.0,
                                            float(GRID - 1), OP.max, OP.min)
                    i0B = posp.tile([128, NI // 16], F32, tag=f"i0B{i}")
                    nc.vector.tensor_scalar(i0B[:], posB[:], MAGIC, MAGIC,
                                            OP.add, OP.subtract)
                    gtB = posp.tile([128, NI // 16], F32, tag=f"gtB{i}")
                    nc.vector.tensor_tensor(gtB[:], i0B[:], posB[:], OP.is_gt)
                    nc.vector.tensor_tensor(i0B[:], i0B[:], gtB[:], OP.subtract)
                    idx = posp.tile([128, NI // 16], I16, tag=f"idx{i}")
                    nc.vector.tensor_copy(idx[:], i0B[:])

                    # gather rows i0,i0+1 (128 floats) per point, in
                    # sub-gathers of SUB indices (descriptor-ring safety)
                    SUB = min(1024, NI)
                    G = gp.tile([128, W * 128], F32, tag=f"G{i}")
                    G3g = G[:].rearrange("p (s e) -> p s e", e=128)
                    for sg in range(NI // SUB):
                        nc.gpsimd.dma_gather(
                            out_ap=G3g[:, sg * (SUB // 128):
                                       (sg + 1) * (SUB // 128), :],
                            in_ap=tab_gap[i],
                            idxs_ap=idx[:, sg * (SUB // 16):
                                        (sg + 1) * (SUB // 16)],
                            num_idxs=SUB,
                            num_idxs_reg=SUB,
                            elem_size=128,
                            elem_step=64,
                        )
                    G3 = G[:].rearrange("p (w e) -> p w e", e=128)
                    G0 = G3[:, :, 0:NCOMP]
                    G1 = G3[:, :, 64:64 + NCOMP]

                    # lerp: v = G0 + (G1-G0)*f
                    d = dp.tile([128, W * NCOMP], F32, tag="d")
                    d3 = d[:].rearrange("p (w c) -> p w c", c=NCOMP)
                    nc.vector.tensor_tensor(d3, G1, G0, OP.subtract)
                    fb = f[:].unsqueeze(2).broadcast_to([128, W, NCOMP])
                    nc.vector.tensor_tensor(d3, d3, fb, OP.mult)
                    v = vp.tile([128, W * NCOMP], F32, tag=f"v{i}")
                    v3 = v[:].rearrange("p (w c) -> p w c", c=NCOMP)
                    nc.vector.tensor_tensor(v3, d3, G0, OP.add)
                    vts.append(v3)

                # plane products, transpose, einsum
                prods = []
                for pi, (a, b) in enumerate(MAT_MODE):
                    prod = prodp.tile([128, W * NCOMP], F32, tag=f"prod{pi}")
                    prod3 = prod[:].rearrange("p (w c) -> p w c", c=NCOMP)
                    nc.vector.tensor_tensor(prod3, vts[a], vts[b], OP.mult)
                    prods.append(prod3)

                outb = outp.tile([128, W * 60], F32, tag="outb")
                ob3 = outb[:].rearrange("p (w d) -> p w d", d=60)
                for w in range(W):
                    prodT = []
                    for pi in range(3):
                        tp = psA.tile([48, 128], F32, tag="tp")
                        nc.tensor.transpose(tp[:], prods[pi][:, w, :], ident[:])
                        sb = ptp.tile([48, 128], F32, tag="prodT")
                        nc.scalar.activation(sb[:], tp[:], AF.Copy)
                        prodT.append(sb)
                    O = psO.tile([128, 60], F32, tag="O")
                    for pi in range(3):
                        nc.tensor.matmul(
                            O[:, 20 * pi:20 * pi + 20],
                            prodT[pi][:],
                            feat_sb[:, 20 * pi:20 * pi + 20],
                            start=True, stop=True,
                        )
                    nc.vector.tensor_scalar(ob3[:, w, :], O[:], float(wq), None,
                                            OP.mult)
                nc.sync.dma_start(
                    outR.ap()[base:base + NI, :].rearrange("(w p) d -> p w d",
                                                           p=128),
                    outb[:].rearrange("p (w d) -> p w d", d=60))

    nc.finalize()
    return nc


# ---------------- entry point ----------------

def kernel(pts, poses, key_poses, tbounds, coord_line, feat_line, part_indices):
    pts = np.asarray(pts, np.float32)
    NP_ = pts.shape[0]
    N = NP_ // N_CORES

    sel, wq = _topk_select(np.asarray(poses, np.float32),
                           np.asarray(key_poses, np.float32),
                           np.asarray(part_indices))
    tabs, featc = _pack_tables(np.asarray(coord_line, np.float32),
                               np.asarray(feat_line, np.float32), sel)
    scales, offsets = _pos_affine(tbounds)

    nc = _build_kernel(N, scales, offsets, wq)

    bc = np.zeros((16, 128), np.float32)
    bc[np.arange(128) % 16, np.arange(128)] = 1.0
    in_maps = []
    for c in range(N_CORES):
        m = {"pts": np.ascontiguousarray(pts[c * N:(c + 1) * N]),
             "featc": featc, "bc16": bc}
        for i in range(3):
            m[f"tab{i}"] = tabs[i]
        in_maps.append(m)

    res = bass_utils.run_bass_kernel_spmd(nc, in_maps,
                                          core_ids=list(range(N_CORES)))
    out = np.concatenate([res.results[c]["outR"] for c in range(N_CORES)], 0)
    return out.reshape(1, NP_, 60).astype(np.float32)
